# revision 28
# baseline (speedup 1.0000x reference)
"""TRN2 Bass kernel for nn_CIFAR10_Type1_Template_Unroll (dense_cnn).

Network (per reference): two locally-connected conv layers + 3-layer FC
head, B=4096. Strategy: pure data parallel over 8 NeuronCores (512 batch
each), activations kept on-chip in [feature, batch] layout, batch N=512
on the matmul free dim throughout.

v3 design notes (from baseline trace analysis):
- Everything fp16 (inputs, weights, activations; PSUM accumulate fp32).
  Measured end-to-end error ~9e-4 vs the 2e-2 gate. Halves DMA bytes.
- The PE clock is HAM-gated: 1.2GHz until ~3.4us of sustained activity,
  re-throttles on idle windows. So: full-array K=128 warmup matmuls on a
  memset tile from t~6us (no DMA dependency), and the L1/L2 emission is
  interleaved at half-row / half-pair-chain granularity so the in-order
  PE queue never head-of-line-waits on PSUM-evac completions.
- PSUM->SBUF evac runs only on ACT + DVE (GPSIMD cannot touch PSUM) at
  ~1 elem/cycle/lane, so evac INSTRUCTIONS are made as large as
  possible: L1 strips pair up in [128,1024] two-bank PSUM tiles (one
  evac per two strips), and an L2 position-pair's two chains share one
  [128,512] bank split by partition range (start=True pending-zero is
  partition-scoped), one evac per pair.
- DMA rides three independent queues: x stream on sync (q1 HWDGE),
  w1/w2/fc2/fc3 on scalar (q10 HWDGE), fc1 on gpsimd (q0 SWDGE), in
  consumption order, large transfers (per-partition lines >= 2KB).
  Buffer-reuse (WAR) hazards are resolved in emission order, so every
  pool allocation is emitted only after the previous tenant's readers.
- L1 (k=2,s=2 locally-connected): host packs per row r a K=32 strip
  (2 positions x 16 feats: 12 real + 4 zero-pad) and block-diagonal
  [32, 128] weight tiles; 4 strips run concurrently via tile_position
  row groups. L2 (k=4,s=2): positions paired on PE col strips 0-63 /
  64-127 via tile_position.
- FC3 is interleaved into the FC2 chain loop (k-major accumulation into
  4 parallel [128,10] PSUM chains) and lands in one [128,40] tile ->
  single output DMA; host undoes the [p, (b4 o)] layout.
"""
import sys

if '/opt/trn_rl_repo' not in sys.path:
    sys.path.insert(0, '/opt/trn_rl_repo')

import numpy as np

N_CORES = 8
BS = 512
WARM_N = 18
LAST_EXEC_NS = None

# ----------------------------------------------------------------- host prep

def _prep_x(x):
    """x [B,3,32,32] -> [N_CORES, 8, 128, 2048] f16 row-pair tiles.

    part = 32*i + 16*q + f; pair p=4g+i covers w1 in {2p,2p+1}; q = w1
    parity; f = c*4 + kh*2 + kw (12..15 zero-pad). Free dim = (g, batch).
    """
    ncr = x.shape[0] // BS
    xr = x.reshape(ncr, BS, 3, 16, 2, 2, 4, 2, 2)   # s,b,c,r,kh,g,i,q,kw
    xt = xr.transpose(0, 3, 5, 6, 7, 2, 4, 8, 1)    # s,r,g,i,q,c,kh,kw,b
    xt = xt.reshape(ncr, 16, 2, 4, 2, 12, BS)
    xpp = np.zeros((ncr, 16, 2, 4, 2, 16, BS), np.float16)
    xpp[..., :12, :] = xt
    # -> s, r, (i,q,f)=128, (g,b)=1024
    xpp = xpp.reshape(ncr, 16, 2, 128, BS).transpose(0, 1, 3, 2, 4)
    xpp = xpp.reshape(ncr, 8, 2, 128, 1024).transpose(0, 1, 3, 2, 4)
    return np.ascontiguousarray(xpp.reshape(ncr, 8, 128, 2048))


def _prep_w1(conv1w):
    """conv1w [64,256,3,2,2] -> [128, 16*256] f16 block-diag strips.

    [p, r*256 + g*128 + c]: strip part p = 32i+16qp+f holds, for parity
    qp, features f -> out channel block c = 64*q + o with q==qp.
    """
    w1r = conv1w.reshape(64, 16, 16, 3, 2, 2)
    wt = w1r.transpose(1, 2, 3, 4, 5, 0).reshape(16, 16, 12, 64)
    wtp = np.zeros((16, 16, 16, 64), np.float32)
    wtp[:, :, :12, :] = wt
    wtp = wtp.reshape(16, 2, 4, 2, 16, 64)          # r,g,i,qp,f,o
    w1t = np.zeros((16, 2, 4, 2, 16, 2, 64), np.float32)
    w1t[:, :, :, 0, :, 0, :] = wtp[:, :, :, 0, :, :]
    w1t[:, :, :, 1, :, 1, :] = wtp[:, :, :, 1, :, :]
    w1t = w1t.reshape(16, 2, 128, 128)              # r,g,p,c
    w1t = w1t.transpose(2, 0, 1, 3)                 # p,r,g,c
    return np.ascontiguousarray(w1t.reshape(128, 16 * 256)).astype(np.float16)


def _h2_posmap():
    pm = np.full((25, 2), -1, np.int64)
    for T in range(21):
        rr, j = divmod(T, 3)
        pm[T, 0] = rr * 7 + 2 * j
        pm[T, 1] = rr * 7 + 2 * j + 1
    for pi in range(4):
        r0, r1 = 2 * pi, 2 * pi + 1
        pm[21 + pi, 0] = r0 * 7 + 6
        if r1 < 7:
            pm[21 + pi, 1] = r1 * 7 + 6
    return pm


# pair-tile consumption order: pass h emits pairs [3h, 3h+1, 3h+2] plus
# cross pairs 21/22/23+24 at passes 2/4/6; w2 DRAM tiles are stored in
# this exact order so each pass is one contiguous DMA.
_W2_ORDER = [0, 1, 2, 3, 4, 5, 6, 7, 8, 21, 9, 10, 11, 12, 13, 14, 22,
             15, 16, 17, 18, 19, 20, 23, 24]
_W2_SLOT = {T: s for s, T in enumerate(_W2_ORDER)}


def _prep_w2(conv2w):
    """conv2w [64,49,64,4,4] -> [25, 128, 1024] f16 pair tiles in
    consumption (_W2_ORDER) order.

    Per position: [128=(q,c), 512=(kh,t,o)]; pair tile free dim =
    (member u, 512).
    """
    w2r = conv2w.reshape(64, 7, 7, 64, 4, 4)
    v = w2r.transpose(1, 2, 3, 4, 5, 0)             # h,w,c,kh,kw,o
    v = v.reshape(7, 7, 64, 4, 2, 2, 64)            # h,w,c,kh,t,q,o
    v = v.transpose(0, 1, 5, 2, 3, 4, 6)            # h,w,q,c,kh,t,o
    pos = v.reshape(49, 128, 512)
    pm = _h2_posmap()
    out = np.zeros((25, 128, 1024), np.float16)
    for T in range(25):
        s = _W2_SLOT[T]
        out[s, :, 0:512] = pos[pm[T, 0]]
        if pm[T, 1] >= 0:
            out[s, :, 512:1024] = pos[pm[T, 1]]
    return np.ascontiguousarray(out)


def _prep_fc1(fc1):
    """fc1 [1024, 3136] -> [8, 128, 3200] f16, k in h2-tile (T) order."""
    pm = _h2_posmap()
    fc1p = fc1.reshape(1024, 64, 49)
    fc1hat = np.zeros((1024, 25, 2, 64), np.float32)
    for T in range(25):
        for u in range(2):
            p = pm[T, u]
            if p >= 0:
                fc1hat[:, T, u, :] = fc1p[:, :, p]
    a = fc1hat.reshape(8, 128, 25, 128).transpose(0, 3, 2, 1)   # m,kp,k,mc
    return np.ascontiguousarray(a.reshape(8, 128, 3200)).astype(np.float16)


def _prep_fc2(fc2):
    """fc2 [512, 1024] -> [128, 4096] f16: [kp, (m k mc)]."""
    a = fc2.reshape(4, 128, 8, 128)                 # m,mc,k,kp
    a = a.transpose(3, 0, 2, 1)                     # kp,m,k,mc
    return np.ascontiguousarray(a.reshape(128, 4096)).astype(np.float16)


def _prep_fc3(fc3):
    """fc3 [10, 512] -> [128, 40] f16: [kp, (k o)]."""
    a = fc3.T.reshape(4, 128, 10)                   # k,kp,o
    a = a.transpose(1, 0, 2)                        # kp,k,o
    return np.ascontiguousarray(a.reshape(128, 40)).astype(np.float16)


# --------------------------------------------------------------- bass kernel

_NC_CACHE = []


def _build_nc():
    import concourse.bass as bass
    import concourse.mybir as mybir
    from concourse import bacc
    from concourse.tile import TileContext

    f32 = mybir.dt.float32
    f16 = mybir.dt.float16
    RELU = mybir.ActivationFunctionType.Relu

    nc = bacc.Bacc("TRN2", target_bir_lowering=False, debug=False,
                   num_devices=N_CORES)
    x_pp = nc.dram_tensor("x_pp", [8, 128, 2048], f16, kind="ExternalInput")
    w1t = nc.dram_tensor("w1t", [128, 4096], f16, kind="ExternalInput")
    w2t = nc.dram_tensor("w2t", [25, 128, 1024], f16, kind="ExternalInput")
    fc1m = nc.dram_tensor("fc1m", [8, 128, 3200], f16, kind="ExternalInput")
    fc2t = nc.dram_tensor("fc2t", [128, 4096], f16, kind="ExternalInput")
    fc3t = nc.dram_tensor("fc3t", [128, 40], f16, kind="ExternalInput")
    y = nc.dram_tensor("y", [10, 512], f32, kind="ExternalOutput")

    pm = _h2_posmap()
    pass_pairs = {h: [3 * h + j for j in range(3)] for h in range(7)}
    pass_pairs[2].append(21)
    pass_pairs[4].append(22)
    pass_pairs[6].extend([23, 24])

    ectr = [0]

    with TileContext(nc) as tc:
        with (
            tc.tile_pool(name="h2pool", bufs=25) as h2pool,
            tc.tile_pool(name="wpool", bufs=4) as wpool,
        ):
            h2 = [h2pool.tile([128, 512], f16, tag="h2", name=f"h2_{T}")
                  for T in range(25)]

            def relu_evac(dst, src):
                if ectr[0] % 2 == 0:
                    nc.scalar.activation(dst, src, RELU)
                else:
                    nc.vector.tensor_scalar_max(dst, src, 0.0)
                ectr[0] += 1

            warm = wpool.tile([128, 512], f16, tag="warm", name="warm",
                              bufs=1)
            nc.vector.memset(warm[:], 0.0)

            # fc1 weight stream (gpsimd SWDGE queue, q0); loads are
            # deferred into the row loop so they don't compete with the
            # latency-critical x/w1 stream at kernel start.
            fc1w = [None] * 8

            def load_fc1(m, gate=None):
                # The scheduler hoists dependency-free DMAs to t~0 where
                # they steal early HBM bandwidth from the x stream. A
                # 1-column dummy copy from a phase-1 tile gives the DMA a
                # real WAW dependency so the transfer starts only after
                # that tile exists.
                wt = wpool.tile([128, 3200], f16, tag="fc1w",
                                name=f"fc1w_{m}", bufs=2)
                if gate is not None:
                    nc.vector.tensor_copy(wt[:, 0:1], gate[:, 0:1])
                nc.gpsimd.dma_start(out=wt[:], in_=fc1m.ap()[m])
                fc1w[m] = wt

            # ---------------- phase 1: L1 + L2 interleaved ----------------
            with (
                tc.tile_pool(name="xp", bufs=4) as xp_pool,
                tc.tile_pool(name="w1p", bufs=2) as w1_pool,
                tc.tile_pool(name="w2p", bufs=3) as w2_pool,
                tc.tile_pool(name="o1p", bufs=40) as o1_pool,
                tc.tile_pool(name="l1ps", bufs=2, space="PSUM") as l1ps,
                tc.tile_pool(name="l2ps", bufs=4, space="PSUM") as l2ps,
            ):
                xt = [None] * 16

                def load_x2(R):
                    # row pair (2R, 2R+1) in one [128, 2048] tile: 4KB
                    # per-partition lines get a 2x bigger share of the
                    # SDMA packet round-robin vs 2KB ones.
                    t = xp_pool.tile([128, 2048], f16, tag="xp",
                                     name=f"xp_{R}")
                    nc.sync.dma_start(out=t[:], in_=x_pp.ap()[R])
                    xt[2 * R] = t[:, 0:1024]
                    xt[2 * R + 1] = t[:, 1024:2048]

                # ALL non-fc1 DMAs issue from the sync engine: a DMA
                # whose WAR dependency has not yet cleared must never sit
                # in front of evacs on ACT/DVE (head-of-line blocks the
                # PSUM recycle and stalls the PE for tens of us).
                w1h = []
                t = w1_pool.tile([128, 2048], f16, tag="w1", name="w1_0")
                nc.sync.dma_start(out=t[:], in_=w1t.ap()[:, 0:2048])
                w1h.append(t)
                for R in range(3):
                    load_x2(R)

                w2tiles = {}

                def load_w2_pass(h):
                    ts = pass_pairs[h]
                    s0 = _W2_SLOT[ts[0]]
                    t = w2_pool.tile([128, 5 * 1024], f16, tag="w2",
                                     name=f"w2p_{h}")
                    if h == 0:
                        nc.sync.dma_start(out=t[:, 0:1024],
                                          in_=w2t.ap()[s0])
                        src = w2t.ap()[s0 + 1:s0 + 3].rearrange(
                            "t p f -> p t f")
                        dst = t[:, 1024:3072].rearrange(
                            "p (t f) -> p t f", t=2)
                        nc.sync.dma_start(out=dst, in_=src)
                    else:
                        src = w2t.ap()[s0:s0 + len(ts)].rearrange(
                            "t p f -> p t f")
                        dst = t[:, 0:1024 * len(ts)].rearrange(
                            "p (t f) -> p t f", t=len(ts))
                        nc.sync.dma_start(out=dst, in_=src)
                    for j, T in enumerate(ts):
                        w2tiles[T] = t[:, 1024 * j:1024 * j + 1024]

                load_w2_pass(0)

                # PE warmup: full-array (K=128, M=128) matmuls on the
                # memset tile so HAM un-throttles during the DMA ramp.
                wps = l2ps.tile([128, 512], f32, tag="l2", name="warm_ps")

                def emit_warm(n):
                    for _ in range(n):
                        nc.tensor.matmul(wps[:], warm[:, 0:128], warm[:],
                                         start=True, stop=True)

                emit_warm(WARM_N)

                out1 = [[None] * 8 for _ in range(16)]

                def emit_l1_group(r, g, half):
                    # tile `half` packs positions {half, half+2}: L2 pair
                    # chains read CONSECUTIVE positions concurrently, so
                    # they must come from different SBUF tiles. A chunk
                    # is emitted between the two halves so each group's
                    # PSUM-recycle (WAR on the previous evac) resolves
                    # under L2 work instead of stalling the PE.
                    w1row = w1h[r // 8][:, 256 * (r % 8):256 * (r % 8) + 256]
                    ps = l1ps.tile([128, 1024], f32, tag="l1",
                                   name=f"l1ps_{r}_{g}_{half}")
                    for sub in range(2):
                        i = half + 2 * sub
                        nc.tensor.matmul(
                            ps[:, 512 * sub:512 * sub + 512],
                            w1row[32 * i:32 * i + 32,
                                  128 * g:128 * g + 128],
                            xt[r][32 * i:32 * i + 32,
                                  512 * g:512 * g + 512],
                            start=True, stop=True,
                            tile_position=(32 * i, 0))
                    ot = o1_pool.tile([128, 1024], f16, tag="o1",
                                      name=f"o1_{r}_{g}_{half}")
                    relu_evac(ot[:], ps[:])
                    for sub in range(2):
                        out1[r][4 * g + half + 2 * sub] = \
                            ot[:, 512 * sub:512 * sub + 512]

                # L2 emission chunks: half-pair-chain granularity.
                chunks = []

                def push_pair(T):
                    # A/B chains share one [128,512] PSUM tile split by
                    # partition range: partitions 0-63 / 64-127 are
                    # physically separate memories so the chains still
                    # run concurrently, and ONE full-partition evac
                    # covers the pair (evac ops have ~790ns fixed cost).
                    pA, pB = pm[T]
                    hA, wA = divmod(int(pA), 7)
                    hB, wB = (None, None) if pB < 0 else divmod(int(pB), 7)
                    wt2 = w2tiles[T]
                    cell = {}

                    def steps(k0, k1):
                        def emit():
                            if k0 == 0:
                                cell['ps'] = l2ps.tile(
                                    [128, 512], f32, tag="l2",
                                    name=f"l2ps_{T}")
                            ps = cell['ps']
                            for kt in range(k0, k1):
                                kh, t = divmod(kt, 2)
                                nc.tensor.matmul(
                                    ps[0:64, :],
                                    wt2[:, 64 * kt:64 * kt + 64],
                                    out1[2 * hA + kh][wA + t],
                                    start=(kt == 0), stop=(kt == 7),
                                    tile_position=(0, 0))
                                if hB is not None:
                                    nc.tensor.matmul(
                                        ps[64:128, :],
                                        wt2[:, 512 + 64 * kt:
                                            512 + 64 * kt + 64],
                                        out1[2 * hB + kh][wB + t],
                                        start=(kt == 0), stop=(kt == 7),
                                        tile_position=(0, 64))
                            if k1 == 8:
                                relu_evac(h2[T][:], ps[:])
                        return emit
                    chunks.append(steps(0, 4))
                    chunks.append(steps(4, 8))

                cpos = [0]

                def emit_chunk():
                    if cpos[0] < len(chunks):
                        chunks[cpos[0]]()
                        cpos[0] += 1

                for r in range(16):
                    for g in range(2):
                        for half in range(2):
                            emit_l1_group(r, g, half)
                            if r >= 3:
                                emit_chunk()
                            else:
                                # keep the PE fed while x streams in
                                emit_warm(2)
                    if r in (1, 3, 5, 7, 9):
                        load_x2((r + 5) // 2)
                    if r == 0:
                        t = w1_pool.tile([128, 2048], f16, tag="w1",
                                         name="w1_1")
                        nc.sync.dma_start(out=t[:],
                                          in_=w1t.ap()[:, 2048:4096])
                        w1h.append(t)
                        load_w2_pass(1)
                        load_fc1(0, gate=out1[0][0])
                    if r == 2:
                        load_w2_pass(2)
                        load_fc1(1, gate=out1[2][0])
                    if r % 2 == 1 and r >= 3:
                        for T in pass_pairs[(r - 3) // 2]:
                            push_pair(T)
                    if r >= 5 and r % 2 == 1 and (r + 1) // 2 <= 6:
                        load_w2_pass((r + 1) // 2)
                    if r == 11:
                        fc2w = wpool.tile([128, 4096], f16, tag="fc2w",
                                          name="fc2w", bufs=1)
                        with tc.tile_wait_until(0.050):
                            nc.sync.dma_start(out=fc2w[:], in_=fc2t.ap())
                            fc3w = wpool.tile([128, 40], f16, tag="fc3w",
                                              name="fc3w", bufs=1)
                            nc.sync.dma_start(out=fc3w[:], in_=fc3t.ap())
                while cpos[0] < len(chunks):
                    emit_chunk()

            # ---------------- phase 2: FC head ----------------
            with (
                tc.tile_pool(name="fcio", bufs=12) as fcio_pool,
                tc.tile_pool(name="fcps", bufs=2, space="PSUM") as fcps,
                tc.tile_pool(name="fc3ps", bufs=1, space="PSUM") as fc3ps,
            ):
                h3 = []
                for m in range(8):
                    wt = fc1w[m]
                    ps = fcps.tile([128, 512], f32, tag="fc",
                                   name=f"fc1ps_{m}")
                    for k in range(25):
                        nc.tensor.matmul(ps[:],
                                         wt[:, 128 * k:128 * k + 128],
                                         h2[k][:],
                                         start=(k == 0), stop=(k == 24))
                    ot = fcio_pool.tile([128, 512], f16, tag="h3",
                                        name=f"h3_{m}", bufs=8)
                    relu_evac(ot[:], ps[:])
                    h3.append(ot)
                    if m < 6:
                        load_fc1(m + 2)

                # FC3 accumulates k-major into one [10, 512] PSUM chain
                # (out = fc3.T slice as lhsT, h4[k] moving), interleaved
                # into the FC2 chain loop; output is y [10, 512], the
                # host transposes back to [512, 10].
                h4 = []
                ps3 = fc3ps.tile([128, 512], f32, tag="fc3", name="fc3ps")

                for m in range(4):
                    ps = fcps.tile([128, 512], f32, tag="fc",
                                   name=f"fc2ps_{m}")
                    for k in range(8):
                        nc.tensor.matmul(
                            ps[:],
                            fc2w[:, 1024 * m + 128 * k:
                                 1024 * m + 128 * k + 128],
                            h3[k][:],
                            start=(k == 0), stop=(k == 7))
                    ot = fcio_pool.tile([128, 512], f16, tag="h4",
                                        name=f"h4_{m}", bufs=4)
                    if m == 3:
                        # last chain feeds the final FC3 matmul: halve
                        # its evac latency by splitting across engines.
                        nc.scalar.activation(ot[:, 0:256], ps[:, 0:256],
                                             RELU)
                        nc.vector.tensor_scalar_max(ot[:, 256:512],
                                                    ps[:, 256:512], 0.0)
                    else:
                        relu_evac(ot[:], ps[:])
                    h4.append(ot)
                    if m >= 1:
                        nc.tensor.matmul(
                            ps3[0:10, :], fc3w[:, 10 * (m - 1):10 * m],
                            h4[m - 1][:],
                            start=(m == 1), stop=False)
                nc.tensor.matmul(ps3[0:10, :], fc3w[:, 30:40], h4[3][:],
                                 start=False, stop=True)

                yt = fcio_pool.tile([128, 512], f32, tag="yt", name="yt",
                                    bufs=1)
                nc.vector.tensor_copy(yt[0:10, :], ps3[0:10, :])
                nc.sync.dma_start(out=y.ap()[:], in_=yt[0:10, :])
    nc.compile()
    return nc


def kernel(x, conv1w, conv2w, fc1, fc2, fc3):
    global LAST_EXEC_NS
    from concourse.bass_utils import run_bass_kernel_spmd

    x = np.ascontiguousarray(np.asarray(x, dtype=np.float32))
    conv1w = np.ascontiguousarray(np.asarray(conv1w, dtype=np.float32))
    conv2w = np.ascontiguousarray(np.asarray(conv2w, dtype=np.float32))
    fc1 = np.ascontiguousarray(np.asarray(fc1, dtype=np.float32))
    fc2 = np.ascontiguousarray(np.asarray(fc2, dtype=np.float32))
    fc3 = np.ascontiguousarray(np.asarray(fc3, dtype=np.float32))

    if not _NC_CACHE:
        _NC_CACHE.append(_build_nc())
    nc = _NC_CACHE[0]

    xpp = _prep_x(x.astype(np.float16))
    shared = {
        "w1t": _prep_w1(conv1w),
        "w2t": _prep_w2(conv2w),
        "fc1m": _prep_fc1(fc1),
        "fc2t": _prep_fc2(fc2),
        "fc3t": _prep_fc3(fc3),
    }
    in_maps = [{**shared, "x_pp": xpp[c]} for c in range(N_CORES)]
    res = run_bass_kernel_spmd(nc, in_maps, list(range(N_CORES)))
    LAST_EXEC_NS = res.exec_time_ns
    # y is [10, 512] per core -> [512, 10]
    outs = [np.ascontiguousarray(r["y"].T) for r in res.results]
    return np.ascontiguousarray(np.concatenate(outs, axis=0))


# revision 29
# speedup vs baseline: 1.0484x; 1.0484x over previous
"""TRN2 Bass kernel for nn_CIFAR10_Type1_Template_Unroll (dense_cnn).

Network (per reference): two locally-connected conv layers + 3-layer FC
head, B=4096. Strategy: pure data parallel over 8 NeuronCores (512 batch
each), activations kept on-chip in [feature, batch] layout, batch N=512
on the matmul free dim throughout.

v3 design notes (from baseline trace analysis):
- Everything fp16 (inputs, weights, activations; PSUM accumulate fp32).
  Measured end-to-end error ~9e-4 vs the 2e-2 gate. Halves DMA bytes.
- The PE clock is HAM-gated: 1.2GHz until ~3.4us of sustained activity,
  re-throttles on idle windows. So: full-array K=128 warmup matmuls on a
  memset tile from t~6us (no DMA dependency), and the L1/L2 emission is
  interleaved at half-row / half-pair-chain granularity so the in-order
  PE queue never head-of-line-waits on PSUM-evac completions.
- PSUM->SBUF evac runs only on ACT + DVE (GPSIMD cannot touch PSUM) at
  ~1 elem/cycle/lane, so evac INSTRUCTIONS are made as large as
  possible: L1 strips pair up in [128,1024] two-bank PSUM tiles (one
  evac per two strips), and an L2 position-pair's two chains share one
  [128,512] bank split by partition range (start=True pending-zero is
  partition-scoped), one evac per pair.
- DMA rides three independent queues: x stream on sync (q1 HWDGE),
  w1/w2/fc2/fc3 on scalar (q10 HWDGE), fc1 on gpsimd (q0 SWDGE), in
  consumption order, large transfers (per-partition lines >= 2KB).
  Buffer-reuse (WAR) hazards are resolved in emission order, so every
  pool allocation is emitted only after the previous tenant's readers.
- L1 (k=2,s=2 locally-connected): host packs per row r a K=32 strip
  (2 positions x 16 feats: 12 real + 4 zero-pad) and block-diagonal
  [32, 128] weight tiles; 4 strips run concurrently via tile_position
  row groups. L2 (k=4,s=2): positions paired on PE col strips 0-63 /
  64-127 via tile_position.
- FC3 is interleaved into the FC2 chain loop (k-major accumulation into
  4 parallel [128,10] PSUM chains) and lands in one [128,40] tile ->
  single output DMA; host undoes the [p, (b4 o)] layout.
"""
import sys

if '/opt/trn_rl_repo' not in sys.path:
    sys.path.insert(0, '/opt/trn_rl_repo')

import numpy as np

N_CORES = 8
BS = 512
WARM_N = 18
LAST_EXEC_NS = None

# ----------------------------------------------------------------- host prep

def _prep_x(x):
    """x [B,3,32,32] -> [N_CORES, 8, 128, 2048] f16 row-pair tiles.

    part = 32*i + 16*q + f; pair p=4g+i covers w1 in {2p,2p+1}; q = w1
    parity; f = c*4 + kh*2 + kw (12..15 zero-pad). Free dim = (g, batch).
    """
    ncr = x.shape[0] // BS
    xr = x.reshape(ncr, BS, 3, 16, 2, 2, 4, 2, 2)   # s,b,c,r,kh,g,i,q,kw
    xt = xr.transpose(0, 3, 5, 6, 7, 2, 4, 8, 1)    # s,r,g,i,q,c,kh,kw,b
    xt = xt.reshape(ncr, 16, 2, 4, 2, 12, BS)
    xpp = np.zeros((ncr, 16, 2, 4, 2, 16, BS), np.float16)
    xpp[..., :12, :] = xt
    # -> s, r, (i,q,f)=128, (g,b)=1024
    xpp = xpp.reshape(ncr, 16, 2, 128, BS).transpose(0, 1, 3, 2, 4)
    xpp = xpp.reshape(ncr, 8, 2, 128, 1024).transpose(0, 1, 3, 2, 4)
    return np.ascontiguousarray(xpp.reshape(ncr, 8, 128, 2048))


def _prep_w1(conv1w):
    """conv1w [64,256,3,2,2] -> [128, 16*256] f16 block-diag strips.

    [p, r*256 + g*128 + c]: strip part p = 32i+16qp+f holds, for parity
    qp, features f -> out channel block c = 64*q + o with q==qp.
    """
    w1r = conv1w.reshape(64, 16, 16, 3, 2, 2)
    wt = w1r.transpose(1, 2, 3, 4, 5, 0).reshape(16, 16, 12, 64)
    wtp = np.zeros((16, 16, 16, 64), np.float32)
    wtp[:, :, :12, :] = wt
    wtp = wtp.reshape(16, 2, 4, 2, 16, 64)          # r,g,i,qp,f,o
    w1t = np.zeros((16, 2, 4, 2, 16, 2, 64), np.float32)
    w1t[:, :, :, 0, :, 0, :] = wtp[:, :, :, 0, :, :]
    w1t[:, :, :, 1, :, 1, :] = wtp[:, :, :, 1, :, :]
    w1t = w1t.reshape(16, 2, 128, 128)              # r,g,p,c
    w1t = w1t.transpose(2, 0, 1, 3)                 # p,r,g,c
    return np.ascontiguousarray(w1t.reshape(128, 16 * 256)).astype(np.float16)


def _h2_posmap():
    pm = np.full((25, 2), -1, np.int64)
    for T in range(21):
        rr, j = divmod(T, 3)
        pm[T, 0] = rr * 7 + 2 * j
        pm[T, 1] = rr * 7 + 2 * j + 1
    for pi in range(4):
        r0, r1 = 2 * pi, 2 * pi + 1
        pm[21 + pi, 0] = r0 * 7 + 6
        if r1 < 7:
            pm[21 + pi, 1] = r1 * 7 + 6
    return pm


# pair-tile consumption order: pass h emits pairs [3h, 3h+1, 3h+2] plus
# cross pairs 21/22/23+24 at passes 2/4/6; w2 DRAM tiles are stored in
# this exact order so each pass is one contiguous DMA.
_W2_ORDER = [0, 1, 2, 3, 4, 5, 6, 7, 8, 21, 9, 10, 11, 12, 13, 14, 22,
             15, 16, 17, 18, 19, 20, 23, 24]
_W2_SLOT = {T: s for s, T in enumerate(_W2_ORDER)}


def _prep_w2(conv2w):
    """conv2w [64,49,64,4,4] -> [25, 128, 1024] f16 pair tiles in
    consumption (_W2_ORDER) order.

    Per position: [128=(q,c), 512=(kh,t,o)]; pair tile free dim =
    (member u, 512).
    """
    w2r = conv2w.reshape(64, 7, 7, 64, 4, 4)
    v = w2r.transpose(1, 2, 3, 4, 5, 0)             # h,w,c,kh,kw,o
    v = v.reshape(7, 7, 64, 4, 2, 2, 64)            # h,w,c,kh,t,q,o
    v = v.transpose(0, 1, 5, 2, 3, 4, 6)            # h,w,q,c,kh,t,o
    pos = v.reshape(49, 128, 512)
    pm = _h2_posmap()
    out = np.zeros((25, 128, 1024), np.float16)
    for T in range(25):
        s = _W2_SLOT[T]
        out[s, :, 0:512] = pos[pm[T, 0]]
        if pm[T, 1] >= 0:
            out[s, :, 512:1024] = pos[pm[T, 1]]
    return np.ascontiguousarray(out)


def _prep_fc1(fc1):
    """fc1 [1024, 3136] -> [8, 128, 3200] f16, k in h2-tile (T) order."""
    pm = _h2_posmap()
    fc1p = fc1.reshape(1024, 64, 49)
    fc1hat = np.zeros((1024, 25, 2, 64), np.float32)
    for T in range(25):
        for u in range(2):
            p = pm[T, u]
            if p >= 0:
                fc1hat[:, T, u, :] = fc1p[:, :, p]
    a = fc1hat.reshape(8, 128, 25, 128).transpose(0, 3, 2, 1)   # m,kp,k,mc
    return np.ascontiguousarray(a.reshape(8, 128, 3200)).astype(np.float16)


def _prep_fc2(fc2):
    """fc2 [512, 1024] -> [128, 4096] f16: [kp, (m k mc)]."""
    a = fc2.reshape(4, 128, 8, 128)                 # m,mc,k,kp
    a = a.transpose(3, 0, 2, 1)                     # kp,m,k,mc
    return np.ascontiguousarray(a.reshape(128, 4096)).astype(np.float16)


def _prep_fc3(fc3):
    """fc3 [10, 512] -> [128, 40] f16: [kp, (k o)]."""
    a = fc3.T.reshape(4, 128, 10)                   # k,kp,o
    a = a.transpose(1, 0, 2)                        # kp,k,o
    return np.ascontiguousarray(a.reshape(128, 40)).astype(np.float16)


# --------------------------------------------------------------- bass kernel

_NC_CACHE = []


def _build_nc():
    import concourse.bass as bass
    import concourse.mybir as mybir
    from concourse import bacc
    from concourse.tile import TileContext

    f32 = mybir.dt.float32
    f16 = mybir.dt.float16
    RELU = mybir.ActivationFunctionType.Relu

    nc = bacc.Bacc("TRN2", target_bir_lowering=False, debug=False,
                   num_devices=N_CORES)
    x_pp = nc.dram_tensor("x_pp", [8, 128, 2048], f16, kind="ExternalInput")
    w1t = nc.dram_tensor("w1t", [128, 4096], f16, kind="ExternalInput")
    w2t = nc.dram_tensor("w2t", [25, 128, 1024], f16, kind="ExternalInput")
    fc1m = nc.dram_tensor("fc1m", [8, 128, 3200], f16, kind="ExternalInput")
    fc2t = nc.dram_tensor("fc2t", [128, 4096], f16, kind="ExternalInput")
    fc3t = nc.dram_tensor("fc3t", [128, 40], f16, kind="ExternalInput")
    y = nc.dram_tensor("y", [10, 512], f32, kind="ExternalOutput")

    pm = _h2_posmap()
    pass_pairs = {h: [3 * h + j for j in range(3)] for h in range(7)}
    pass_pairs[2].append(21)
    pass_pairs[4].append(22)
    pass_pairs[6].extend([23, 24])

    ectr = [0]

    with TileContext(nc) as tc:
        with (
            tc.tile_pool(name="h2pool", bufs=25) as h2pool,
            tc.tile_pool(name="wpool", bufs=4) as wpool,
        ):
            h2 = [h2pool.tile([128, 512], f16, tag="h2", name=f"h2_{T}")
                  for T in range(25)]

            def relu_evac(dst, src):
                if ectr[0] % 2 == 0:
                    nc.scalar.activation(dst, src, RELU)
                else:
                    nc.vector.tensor_scalar_max(dst, src, 0.0)
                ectr[0] += 1

            warm = wpool.tile([128, 512], f16, tag="warm", name="warm",
                              bufs=1)
            nc.vector.memset(warm[:], 0.0)

            # fc1 weight stream (gpsimd SWDGE queue, q0); loads are
            # deferred into the row loop so they don't compete with the
            # latency-critical x/w1 stream at kernel start.
            fc1w = [None] * 8

            def load_fc1(m, gate=None):
                # The scheduler hoists dependency-free DMAs to t~0 where
                # they steal early HBM bandwidth from the x stream. A
                # 1-column dummy copy from a phase-1 tile gives the DMA a
                # real WAW dependency so the transfer starts only after
                # that tile exists.
                wt = wpool.tile([128, 3200], f16, tag="fc1w",
                                name=f"fc1w_{m}", bufs=2)
                if gate is not None:
                    nc.vector.tensor_copy(wt[:, 0:1], gate[:, 0:1])
                nc.gpsimd.dma_start(out=wt[:], in_=fc1m.ap()[m])
                fc1w[m] = wt

            # ---------------- phase 1: L1 + L2 interleaved ----------------
            with (
                tc.tile_pool(name="xp", bufs=4) as xp_pool,
                tc.tile_pool(name="w1p", bufs=2) as w1_pool,
                tc.tile_pool(name="w2p", bufs=3) as w2_pool,
                tc.tile_pool(name="o1p", bufs=40) as o1_pool,
                tc.tile_pool(name="l1ps", bufs=2, space="PSUM") as l1ps,
                tc.tile_pool(name="l2ps", bufs=4, space="PSUM") as l2ps,
            ):
                xt = [None] * 16

                def load_x2(R):
                    # row pair (2R, 2R+1) in one [128, 2048] tile: 4KB
                    # per-partition lines get a 2x bigger share of the
                    # SDMA packet round-robin vs 2KB ones.
                    t = xp_pool.tile([128, 2048], f16, tag="xp",
                                     name=f"xp_{R}")
                    nc.sync.dma_start(out=t[:], in_=x_pp.ap()[R])
                    xt[2 * R] = t[:, 0:1024]
                    xt[2 * R + 1] = t[:, 1024:2048]

                # ALL non-fc1 DMAs issue from the sync engine: a DMA
                # whose WAR dependency has not yet cleared must never sit
                # in front of evacs on ACT/DVE (head-of-line blocks the
                # PSUM recycle and stalls the PE for tens of us).
                w1h = []
                t = w1_pool.tile([128, 2048], f16, tag="w1", name="w1_0")
                nc.sync.dma_start(out=t[:], in_=w1t.ap()[:, 0:2048])
                w1h.append(t)
                for R in range(3):
                    load_x2(R)

                w2tiles = {}

                def load_w2_pass(h):
                    ts = pass_pairs[h]
                    s0 = _W2_SLOT[ts[0]]
                    t = w2_pool.tile([128, 5 * 1024], f16, tag="w2",
                                     name=f"w2p_{h}")
                    if h == 0:
                        nc.sync.dma_start(out=t[:, 0:1024],
                                          in_=w2t.ap()[s0])
                        src = w2t.ap()[s0 + 1:s0 + 3].rearrange(
                            "t p f -> p t f")
                        dst = t[:, 1024:3072].rearrange(
                            "p (t f) -> p t f", t=2)
                        nc.sync.dma_start(out=dst, in_=src)
                    else:
                        src = w2t.ap()[s0:s0 + len(ts)].rearrange(
                            "t p f -> p t f")
                        dst = t[:, 0:1024 * len(ts)].rearrange(
                            "p (t f) -> p t f", t=len(ts))
                        nc.sync.dma_start(out=dst, in_=src)
                    for j, T in enumerate(ts):
                        w2tiles[T] = t[:, 1024 * j:1024 * j + 1024]

                load_w2_pass(0)

                # PE warmup: full-array (K=128, M=128) matmuls on the
                # memset tile so HAM un-throttles during the DMA ramp.
                wps = l2ps.tile([128, 512], f32, tag="l2", name="warm_ps")

                def emit_warm(n):
                    for _ in range(n):
                        nc.tensor.matmul(wps[:], warm[:, 0:128], warm[:],
                                         start=True, stop=True)

                emit_warm(WARM_N)

                out1 = [[None] * 8 for _ in range(16)]

                def emit_l1_group(r, g, half):
                    # tile `half` packs positions {half, half+2}: L2 pair
                    # chains read CONSECUTIVE positions concurrently, so
                    # they must come from different SBUF tiles. A chunk
                    # is emitted between the two halves so each group's
                    # PSUM-recycle (WAR on the previous evac) resolves
                    # under L2 work instead of stalling the PE.
                    w1row = w1h[r // 8][:, 256 * (r % 8):256 * (r % 8) + 256]
                    ps = l1ps.tile([128, 1024], f32, tag="l1",
                                   name=f"l1ps_{r}_{g}_{half}")
                    for sub in range(2):
                        i = half + 2 * sub
                        nc.tensor.matmul(
                            ps[:, 512 * sub:512 * sub + 512],
                            w1row[32 * i:32 * i + 32,
                                  128 * g:128 * g + 128],
                            xt[r][32 * i:32 * i + 32,
                                  512 * g:512 * g + 512],
                            start=True, stop=True,
                            tile_position=(32 * i, 0))
                    ot = o1_pool.tile([128, 1024], f16, tag="o1",
                                      name=f"o1_{r}_{g}_{half}")
                    relu_evac(ot[:], ps[:])
                    for sub in range(2):
                        out1[r][4 * g + half + 2 * sub] = \
                            ot[:, 512 * sub:512 * sub + 512]

                # L2 emission chunks: half-pair-chain granularity.
                chunks = []

                def push_pair(T):
                    # A/B chains share one [128,512] PSUM tile split by
                    # partition range: partitions 0-63 / 64-127 are
                    # physically separate memories so the chains still
                    # run concurrently, and ONE full-partition evac
                    # covers the pair (evac ops have ~790ns fixed cost).
                    pA, pB = pm[T]
                    hA, wA = divmod(int(pA), 7)
                    hB, wB = (None, None) if pB < 0 else divmod(int(pB), 7)
                    wt2 = w2tiles[T]
                    cell = {}

                    def steps(k0, k1):
                        def emit():
                            if k0 == 0:
                                cell['ps'] = l2ps.tile(
                                    [128, 512], f32, tag="l2",
                                    name=f"l2ps_{T}")
                            ps = cell['ps']
                            for kt in range(k0, k1):
                                kh, t = divmod(kt, 2)
                                nc.tensor.matmul(
                                    ps[0:64, :],
                                    wt2[:, 64 * kt:64 * kt + 64],
                                    out1[2 * hA + kh][wA + t],
                                    start=(kt == 0), stop=(kt == 7),
                                    tile_position=(0, 0))
                                if hB is not None:
                                    nc.tensor.matmul(
                                        ps[64:128, :],
                                        wt2[:, 512 + 64 * kt:
                                            512 + 64 * kt + 64],
                                        out1[2 * hB + kh][wB + t],
                                        start=(kt == 0), stop=(kt == 7),
                                        tile_position=(0, 64))
                            if k1 == 8:
                                relu_evac(h2[T][:], ps[:])
                        return emit
                    chunks.append(steps(0, 4))
                    chunks.append(steps(4, 8))

                cpos = [0]

                def emit_chunk():
                    if cpos[0] < len(chunks):
                        chunks[cpos[0]]()
                        cpos[0] += 1

                for r in range(16):
                    for g in range(2):
                        emit_l1_group(r, g, 0)
                        emit_l1_group(r, g, 1)
                        if r >= 3:
                            emit_chunk()
                            emit_chunk()
                        else:
                            # keep the PE fed while x streams in
                            emit_warm(3)
                    if r in (1, 3, 5, 7, 9):
                        load_x2((r + 5) // 2)
                    if r == 0:
                        t = w1_pool.tile([128, 2048], f16, tag="w1",
                                         name="w1_1")
                        nc.sync.dma_start(out=t[:],
                                          in_=w1t.ap()[:, 2048:4096])
                        w1h.append(t)
                        load_w2_pass(1)
                        load_fc1(0, gate=out1[0][0])
                    if r == 2:
                        load_w2_pass(2)
                        load_fc1(1, gate=out1[2][0])
                    if r % 2 == 1 and r >= 3:
                        for T in pass_pairs[(r - 3) // 2]:
                            push_pair(T)
                    if r >= 5 and r % 2 == 1 and (r + 1) // 2 <= 6:
                        load_w2_pass((r + 1) // 2)
                    if r == 11:
                        fc2w = wpool.tile([128, 4096], f16, tag="fc2w",
                                          name="fc2w", bufs=1)
                        with tc.tile_wait_until(0.050):
                            nc.sync.dma_start(out=fc2w[:], in_=fc2t.ap())
                            fc3w = wpool.tile([128, 40], f16, tag="fc3w",
                                              name="fc3w", bufs=1)
                            nc.sync.dma_start(out=fc3w[:], in_=fc3t.ap())
                while cpos[0] < len(chunks):
                    emit_chunk()

            # ---------------- phase 2: FC head ----------------
            with (
                tc.tile_pool(name="fcio", bufs=12) as fcio_pool,
                tc.tile_pool(name="fcps", bufs=2, space="PSUM") as fcps,
                tc.tile_pool(name="fc3ps", bufs=1, space="PSUM") as fc3ps,
            ):
                h3 = []
                for m in range(8):
                    wt = fc1w[m]
                    ps = fcps.tile([128, 512], f32, tag="fc",
                                   name=f"fc1ps_{m}")
                    for k in range(25):
                        nc.tensor.matmul(ps[:],
                                         wt[:, 128 * k:128 * k + 128],
                                         h2[k][:],
                                         start=(k == 0), stop=(k == 24))
                    ot = fcio_pool.tile([128, 512], f16, tag="h3",
                                        name=f"h3_{m}", bufs=8)
                    relu_evac(ot[:], ps[:])
                    h3.append(ot)
                    if m < 6:
                        load_fc1(m + 2)

                # FC3 accumulates k-major into one [10, 512] PSUM chain
                # (out = fc3.T slice as lhsT, h4[k] moving), interleaved
                # into the FC2 chain loop; output is y [10, 512], the
                # host transposes back to [512, 10].
                h4 = []
                ps3 = fc3ps.tile([128, 512], f32, tag="fc3", name="fc3ps")

                for m in range(4):
                    ps = fcps.tile([128, 512], f32, tag="fc",
                                   name=f"fc2ps_{m}")
                    for k in range(8):
                        nc.tensor.matmul(
                            ps[:],
                            fc2w[:, 1024 * m + 128 * k:
                                 1024 * m + 128 * k + 128],
                            h3[k][:],
                            start=(k == 0), stop=(k == 7))
                    ot = fcio_pool.tile([128, 512], f16, tag="h4",
                                        name=f"h4_{m}", bufs=4)
                    if m == 3:
                        # last chain feeds the final FC3 matmul: halve
                        # its evac latency by splitting across engines.
                        nc.scalar.activation(ot[:, 0:256], ps[:, 0:256],
                                             RELU)
                        nc.vector.tensor_scalar_max(ot[:, 256:512],
                                                    ps[:, 256:512], 0.0)
                    else:
                        relu_evac(ot[:], ps[:])
                    h4.append(ot)
                    if m >= 1:
                        nc.tensor.matmul(
                            ps3[0:10, :], fc3w[:, 10 * (m - 1):10 * m],
                            h4[m - 1][:],
                            start=(m == 1), stop=False)
                nc.tensor.matmul(ps3[0:10, :], fc3w[:, 30:40], h4[3][:],
                                 start=False, stop=True)

                yt = fcio_pool.tile([128, 512], f32, tag="yt", name="yt",
                                    bufs=1)
                nc.vector.tensor_copy(yt[0:10, :], ps3[0:10, :])
                nc.sync.dma_start(out=y.ap()[:], in_=yt[0:10, :])
    nc.compile()
    return nc


def kernel(x, conv1w, conv2w, fc1, fc2, fc3):
    global LAST_EXEC_NS
    from concourse.bass_utils import run_bass_kernel_spmd

    x = np.ascontiguousarray(np.asarray(x, dtype=np.float32))
    conv1w = np.ascontiguousarray(np.asarray(conv1w, dtype=np.float32))
    conv2w = np.ascontiguousarray(np.asarray(conv2w, dtype=np.float32))
    fc1 = np.ascontiguousarray(np.asarray(fc1, dtype=np.float32))
    fc2 = np.ascontiguousarray(np.asarray(fc2, dtype=np.float32))
    fc3 = np.ascontiguousarray(np.asarray(fc3, dtype=np.float32))

    if not _NC_CACHE:
        _NC_CACHE.append(_build_nc())
    nc = _NC_CACHE[0]

    xpp = _prep_x(x.astype(np.float16))
    shared = {
        "w1t": _prep_w1(conv1w),
        "w2t": _prep_w2(conv2w),
        "fc1m": _prep_fc1(fc1),
        "fc2t": _prep_fc2(fc2),
        "fc3t": _prep_fc3(fc3),
    }
    in_maps = [{**shared, "x_pp": xpp[c]} for c in range(N_CORES)]
    res = run_bass_kernel_spmd(nc, in_maps, list(range(N_CORES)))
    LAST_EXEC_NS = res.exec_time_ns
    # y is [10, 512] per core -> [512, 10]
    outs = [np.ascontiguousarray(r["y"].T) for r in res.results]
    return np.ascontiguousarray(np.concatenate(outs, axis=0))
